# revision 5
# baseline (speedup 1.0000x reference)
"""GAT attention layer (nn_AttentionLayer) on 8 Trainium2 NeuronCores.

Row-sharded: core c owns rows I_c = [c*N/8, (c+1)*N/8) of `features`/`adj`
and computes out[I_c, :].

Math (per reference):
    h = X @ W                                  [N, F]
    s1 = h @ a1, s2 = h @ a2                   [N]
    e_ij = leaky_relu(s1_i + s2_j, 0.2)
    P = softmax_j(where(adj>0, e, -inf))
    out = elu(P @ h)

Device algorithm per core (fp16 elementwise, fp32 accumulation):
    - local h/s1/s2 from a tiny fp32 matmul  (X^T via PE transpose)
    - one AllGather of [h_fp16 | s2_fp16]
    - per 128-row i-tile:
        adj row-block DMA-loaded with int32->fp16 cast (SWDGE)
        m   = adj*BIG + (s1 - BIG)        (DVE tensor_scalar, 4x)
        x   = m + s2_bcast                (DVE tensor_tensor, 2x)
        y   = max(x, 0.2*x)               (DVE scalar_tensor_tensor, 2x)
        P   = exp(y - 4)                  (ACT, exact 0 for masked via fp16 underflow)
        P^T tiles via batched DMA xbar transpose (SBUF->SBUF, fp16)
        psum[i, 0:F]  += P^T.T @ [h | 1]  (PE fp16, fp32 accum; col F = row sums)
        out = elu(psum[:, :F] / psum[:, F])
"""

import os
import sys

for _p in ("/opt/trn_rl_repo",):
    if os.path.isdir(_p) and _p not in sys.path:
        sys.path.append(_p)

import numpy as np

import concourse.bass as bass
import concourse.bacc as bacc
import concourse.mybir as mybir
import concourse.tile as tile
from concourse import bass_utils

N, D, F = 8192, 256, 64
NCORES = 8
RL = N // NCORES          # rows per core
BIG = 240.0
ALPHA = 0.2
CSHIFT = 4.0

f32 = mybir.dt.float32
fp16 = mybir.dt.float16
i32 = mybir.dt.int32
Alu = mybir.AluOpType
Act = mybir.ActivationFunctionType

LAST_RESULTS = None  # BassKernelResults of the most recent kernel() call
_CACHE = {}


def _kernel_body(tc, out_d, feat_d, adj_d, W_d, a_d, n=N, rl=RL, ncores=NCORES):
    """Builds the per-core program. n = global rows, rl = local rows."""
    nc = tc.nc
    nit = rl // 128           # i-tiles per core
    njt = n // 128            # j-tiles (global)
    nk = D // 128             # contraction tiles for h matmul

    ident_d = nc.inline_tensor(np.eye(128, dtype=np.float32), name="ident128")

    # collective bounce buffers; payload layout: [ h fp16 (rl*F) | s2 fp16 (rl) ]
    cc_in = nc.dram_tensor("cc_in", [rl * (F + 1)], fp16, kind="Internal").ap()
    cc_out = nc.dram_tensor(
        "cc_out", [n * (F + 1)], fp16, kind="Internal", addr_space="Shared"
    ).ap()

    with (
        tc.tile_pool(name="sbP", bufs=1) as sbP,          # persistent tiles
        tc.tile_pool(name="sbX", bufs=2) as sbX,          # X load staging
        tc.tile_pool(name="sbA", bufs=3) as sbA,          # adj tiles
        tc.tile_pool(name="sbW", bufs=3) as sbW,          # work (m/x/y/P) tiles
        tc.tile_pool(name="sbT", bufs=2) as sbT,          # P^T tiles
        tc.tile_pool(name="sbE", bufs=4) as sbE,          # epilogue scratch
        tc.tile_pool(name="ppB", bufs=2, space="PSUM") as ppB,   # phase B psum
        tc.tile_pool(name="ppO", bufs=3, space="PSUM") as ppO,   # output psum
    ):
        # ---------------- adj stream head-start (no dependencies) ----------
        adjf = [sbA.tile([128, n], fp16, tag="adjf", name=f"adjf{i}") for i in range(nit)]
        for it in range(min(3, nit)):
            nc.gpsimd.dma_start(adjf[it][:], adj_d[it * 128 : (it + 1) * 128, :])

        # ---------------- phase A: constants ------------------------------
        ident = sbP.tile([128, 128], f32)
        nc.sync.dma_start(ident[:], ident_d.ap())
        cshift = sbP.tile([128, 1], f32)
        nc.vector.memset(cshift[:], -CSHIFT)
        arow = sbP.tile([1, 2 * F], f32)
        nc.sync.dma_start(arow[:], a_d.rearrange("f o -> o f"))
        ab = sbP.tile([128, 2 * F], f32)
        nc.gpsimd.partition_broadcast(ab[:], arow[:])
        wsb = sbP.tile([128, nk, F], f32)
        nc.sync.dma_start(wsb[:], W_d.rearrange("(k p) f -> p k f", p=128))
        # rhs_small[:, k, :] = [ W_k | wa1_k | wa2_k ]
        rhs_small = sbP.tile([128, nk, F + 2], f32)
        scr = sbP.tile([128, F], f32)
        for k in range(nk):
            nc.vector.tensor_copy(rhs_small[:, k, :F], wsb[:, k, :])
            nc.vector.scalar_tensor_tensor(
                scr[:], wsb[:, k, :], 1.0, ab[:, :F], Alu.mult, Alu.mult,
                accum_out=rhs_small[:, k, F : F + 1],
            )
            nc.vector.scalar_tensor_tensor(
                scr[:], wsb[:, k, :], 1.0, ab[:, F:], Alu.mult, Alu.mult,
                accum_out=rhs_small[:, k, F + 1 : F + 2],
            )

        # ---------------- phase B: X^T, h, s1, s2 -------------------------
        xT = sbP.tile([128, nk, rl], f32)
        for it in range(nit):
            xin = sbX.tile([128, D], f32, tag="xin")
            nc.sync.dma_start(xin[:], feat_d[it * 128 : (it + 1) * 128, :])
            for k in range(nk):
                pst = ppB.tile([128, 128], f32, tag="pst")
                nc.tensor.transpose(pst[:], xin[:, k * 128 : (k + 1) * 128], ident[:])
                nc.vector.tensor_copy(xT[:, k, it * 128 : (it + 1) * 128], pst[:])

        hs16 = sbP.tile([128, nit, F], fp16)     # local h (fp16)
        s2l16 = sbP.tile([128, nit], fp16)       # local s2 (fp16)
        s1mB = sbP.tile([128, nit], f32)         # s1 - BIG
        for it in range(nit):
            psh = ppB.tile([128, F + 2], f32, tag="psh")
            for k in range(nk):
                nc.tensor.matmul(
                    psh[:],
                    xT[:, k, it * 128 : (it + 1) * 128],
                    rhs_small[:, k, :],
                    start=(k == 0),
                    stop=(k == nk - 1),
                )
            nc.vector.tensor_copy(hs16[:, it, :], psh[:, :F])
            nc.vector.tensor_scalar_add(s1mB[:, it : it + 1], psh[:, F : F + 1], -BIG)
            nc.vector.tensor_copy(s2l16[:, it : it + 1], psh[:, F + 1 : F + 2])

        # ---------------- collective: AllGather [h | s2] -------------------
        nc.sync.dma_start(
            cc_in[: rl * F].rearrange("(t p f) -> p t f", p=128, f=F), hs16[:]
        )
        nc.sync.dma_start(
            cc_in[rl * F :].rearrange("(t p) -> p t", p=128), s2l16[:]
        )
        nc.gpsimd.collective_compute(
            "AllGather",
            Alu.bypass,
            replica_groups=[list(range(ncores))],
            ins=[cc_in.opt()],
            outs=[cc_out.opt()],
        )

        # s2 broadcast [128, n] fp16, straight from DRAM with stride-0 partitions
        cc_ranks = cc_out.rearrange("(r y) -> r y", r=ncores)
        s2_part = cc_ranks[:, rl * F :]          # [ncores, rl]
        s2b = sbP.tile([128, n], fp16)
        nc.gpsimd.dma_start(s2b[:], s2_part.unsqueeze(0).partition_broadcast(128))

        # h with ones column: hsb[p, t, 0:F] = h[t*128+p, :], hsb[p, t, F] = 1
        hsb = sbP.tile([128, njt, F + 1], fp16)
        for r in range(ncores):
            nc.sync.dma_start(
                hsb[:, r * nit : (r + 1) * nit, :F],
                cc_ranks[r, : rl * F].rearrange("(t p f) -> p t f", p=128, f=F),
            )
        nc.vector.memset(hsb[:, :, F : F + 1], 1.0)

        # ---------------- phase C: attention rows -------------------------
        work = [sbW.tile([128, n], fp16, tag="work", name=f"work{i}") for i in range(nit)]
        pt = [sbT.tile([128, njt, 128], fp16, tag="pt", name=f"pt{i}") for i in range(nit)]
        pso = [ppO.tile([128, F + 1], f32, tag="pso", name=f"pso{i}") for i in range(nit)]

        def epilogue(it):
            ps = pso[it]
            rcp = sbE.tile([128, 1], f32, tag="rcp")
            nc.vector.reciprocal(rcp[:], ps[:, F : F + 1])
            o = sbE.tile([128, F], f32, tag="o")
            nc.vector.tensor_scalar_mul(o[:], ps[:, :F], rcp[:])
            q = sbE.tile([128, F], f32, tag="q")
            nc.vector.tensor_scalar_min(q[:], o[:], 0.0)
            e = sbE.tile([128, F], f32, tag="e")
            nc.scalar.activation(e[:], q[:], Act.Exp)
            r = sbE.tile([128, F], f32, tag="r")
            nc.vector.tensor_scalar_max(r[:], o[:], 0.0)
            fin = sbE.tile([128, F], f32, tag="fin")
            nc.vector.scalar_tensor_tensor(
                fin[:], e[:], -1.0, r[:], Alu.add, Alu.add
            )
            nc.sync.dma_start(out_d[it * 128 : (it + 1) * 128, :], fin[:])

        for it in range(nit):
            if it >= 3:
                nc.gpsimd.dma_start(adjf[it][:], adj_d[it * 128 : (it + 1) * 128, :])
            w = work[it]
            nc.vector.tensor_scalar(
                w[:], adjf[it][:], BIG, s1mB[:, it : it + 1], Alu.mult, Alu.add
            )
            nc.vector.tensor_tensor(w[:], w[:], s2b[:], Alu.add)
            nc.vector.scalar_tensor_tensor(
                w[:], w[:], ALPHA, w[:], Alu.mult, Alu.max
            )
            nc.scalar.activation(w[:], w[:], Act.Exp, bias=cshift[:], scale=1.0)
            nc.sync.dma_start_transpose(pt[it][:], w[:])
            for t in range(njt):
                nc.tensor.matmul(
                    pso[it][:],
                    pt[it][:, t, :],
                    hsb[:, t, :],
                    start=(t == 0),
                    stop=(t == njt - 1),
                )
            if it >= 2:
                epilogue(it - 2)
        for j in range(max(0, nit - 2), nit):
            epilogue(j)


def _build(n=N, rl=RL, ncores=NCORES):
    key = (n, rl, ncores)
    if key in _CACHE:
        return _CACHE[key]
    nc = bacc.Bacc(
        "TRN2", target_bir_lowering=False, debug=False, num_devices=ncores
    )
    feat = nc.dram_tensor("features", [rl, D], f32, kind="ExternalInput").ap()
    adj = nc.dram_tensor("adj", [rl, n], i32, kind="ExternalInput").ap()
    W = nc.dram_tensor("W", [D, F], f32, kind="ExternalInput").ap()
    a = nc.dram_tensor("a", [2 * F, 1], f32, kind="ExternalInput").ap()
    out = nc.dram_tensor("out", [rl, F], f32, kind="ExternalOutput").ap()
    with tile.TileContext(nc) as tc:
        _kernel_body(tc, out, feat, adj, W, a, n=n, rl=rl, ncores=ncores)
    nc.compile()
    _CACHE[key] = nc
    return nc


def kernel(features, adj, W, a):
    global LAST_RESULTS
    features = np.ascontiguousarray(features, dtype=np.float32)
    adj = np.ascontiguousarray(adj, dtype=np.int32)
    W = np.ascontiguousarray(W, dtype=np.float32)
    a = np.ascontiguousarray(a, dtype=np.float32)

    n = adj.shape[0]
    rl = n // NCORES
    nc = _build(n=n, rl=rl, ncores=NCORES)
    in_maps = [
        {
            "features": features[c * rl : (c + 1) * rl],
            "adj": adj[c * rl : (c + 1) * rl],
            "W": W,
            "a": a,
        }
        for c in range(NCORES)
    ]
    res = bass_utils.run_bass_kernel_spmd(nc, in_maps, core_ids=list(range(NCORES)))
    LAST_RESULTS = res
    return np.concatenate([res.results[c]["out"] for c in range(NCORES)], axis=0)
